# revision 1
# baseline (speedup 1.0000x reference)
"""Trainium2 Bass kernel for nn_MemResProjections (memory-residual attention).

Reference computation (B=4, S=2048, K=64, H=1024, fp32):
    normed = rmsnorm(hidden) * norm_w
    v_pool = concat([normed, memory], axis=1)            # (B, S+K, H)
    q = normed @ q_w.T ; k = v_pool @ k_w.T
    logits = q @ k.T / sqrt(H)  with causal mask on the local S block,
    memory columns fully visible
    attn = softmax(logits); h~ = attn @ v_pool
    alpha = sigmoid(hidden @ gate_w.T + gate_b)
    out = (1-alpha)*hidden + alpha*h~

Sharding: 8 cores = (batch b, half h) pairs; each core owns 1024 query rows.
Cores with h=1 see the first 1024 rows as a fully-visible "prefix"; cores with
h=0 get a zero prefix that is masked out via a per-core additive bias fused
into the exp() activation, keeping the SPMD program uniform.

Per-core dataflow (all matmuls in float32r = full-rate fp32, ~13-bit mantissa):
  A1: rmsnorm stats in natural layout; normed stripes PE-transposed into
      normedT (h on partitions); v = normed*norm_w spilled to DRAM scratch.
  A2: gate logits from normedT with the rstd factored out (gate uses raw x =
      normed * std, so scale the psum by std on eviction); sigmoid -> alpha
      spilled to DRAM.
  A3: qT = (q_w*norm_w).T-stationary @ normedT ; kT over own+prefix+memory.
  B:  scores^T tiles (t on partitions, 256 query cols) -> fused scale+mask+exp
      -> h~ accumulation (lhsT=exp^T) + denominator via ones-matmul; the
      [1,256] denominator hops to [128,2] natural layout with one SBUF DMA;
      output = x + alpha*(h~/den - x).
"""
import numpy as np

import concourse.bass as bass
import concourse.mybir as mybir
import concourse.tile as tile
from concourse.bass_utils import run_bass_kernel_spmd

F32 = mybir.dt.float32
F32R = mybir.dt.float32r
AFT = mybir.ActivationFunctionType

P = 128
H = 1024
S_OWN = 1024          # query rows per core
T_PREF = 1024         # prefix kv rows
T_MEM = 64            # memory kv rows
NJ = H // P           # h blocks
NS = S_OWN // P       # own stripes
SCALE = 1.0 / 32.0    # 1/sqrt(H)
EPS = 1e-6
NEG = -1.0e30

N_CORES = 8
B_FULL, S_FULL = 4, 2048


# ---------------------------------------------------------------- walrus fix
ENGINE_ATTR = {
    mybir.EngineType.PE: "tensor",
    mybir.EngineType.Activation: "scalar",
    mybir.EngineType.DVE: "vector",
    mybir.EngineType.Pool: "gpsimd",
    mybir.EngineType.SP: "sync",
}
DMA_OPS = ("InstDMACopy", "InstDMATranspose", "InstTensorLoad", "InstTensorSave",
           "InstCollectiveCompute")


def split_multi_waits(nc, limit=1, dma_limit=None):
    """This walrus build rejects engine instructions carrying more than one
    sem wait; hoist extras onto same-engine NOPs inserted just before."""
    n_split = 0
    for f in nc.m.functions:
        for blk in f.blocks:
            il = blk.instructions
            i = 0
            while i < len(il):
                ins = il[i]
                is_dma = type(ins).__name__ in DMA_OPS
                lim = dma_limit if is_dma else limit
                si = ins.sync_info
                waits = list(si.on_wait) if si is not None and si.on_wait else []
                if lim is not None and len(waits) > lim:
                    keep, extra = waits[:lim], waits[lim:]
                    si.on_wait.clear()
                    for w in keep:
                        si.on_wait.append(w)
                    eng = getattr(nc, ENGINE_ATTR[ins.engine])
                    for w in extra:
                        nop = eng.nop(nofuse=True, hint="wait_split")
                        nop.wait_op(bass.SemaphoreHandle(w.ant_name, w.id),
                                    w.wait_value, "sem-ge")
                        popped = nc.cur_bb.bb.instructions.pop()
                        assert popped.name == nop.ins.name
                        il.insert(i, nop.ins)
                        i += 1
                        n_split += 1
                i += 1
    return n_split


# ---------------------------------------------------------------- program
def build_nc():
    nc = bass.Bass()
    dp = lambda n, shp: nc.declare_dram_parameter(n, shp, F32, isOutput=False)
    x_own = dp("x_own", [S_OWN, H])
    x_pref = dp("x_pref", [T_PREF, H])
    mem = dp("mem", [T_MEM, H])
    memT = dp("memT", [H, T_MEM])
    qwT = dp("qwT", [H, H])        # (q_w * norm_w).T
    kwT = dp("kwT", [H, H])        # (k_w * norm_w).T
    gwT = dp("gwT", [H, H])        # gate_w.T
    w_bc = dp("w_bc", [P, H])      # norm_w broadcast
    b_bc = dp("b_bc", [P, H])      # gate_b broadcast
    pbias = dp("pbias", [P, 1])    # 0 (h=1) or -1e30 (h=0)
    onesc = dp("onesc", [P, 1])    # ones column (fp32r matmul operand)
    tri = dp("tri", [P, P])        # additive mask: 0 if col>=row else -1e30
    ident = dp("ident", [P, P])    # identity for PE transpose
    out = nc.declare_dram_parameter("out", [S_OWN, H], F32, isOutput=True)
    import os as _os
    DEBUG = _os.environ.get("DEBUG_KERNEL") == "1"
    if DEBUG:
        dbg_rden = nc.declare_dram_parameter("dbg_rden", [P, NS], F32,
                                             isOutput=True)

    v_dram = nc.dram_tensor("v_scratch", [S_OWN + T_PREF, H], F32)
    a_dram = nc.dram_tensor("alpha_scratch", [S_OWN, H], F32)

    T_ALL = S_OWN + T_PREF  # spill rows

    with tile.TileContext(nc) as tc:
        from contextlib import ExitStack
        with ExitStack() as ctx:
            # ---- long-lived pools
            const = ctx.enter_context(tc.tile_pool(name="const", bufs=1))
            proj = ctx.enter_context(tc.tile_pool(name="proj", bufs=1))

            eps_t = const.tile([P, 1], F32)
            nc.vector.memset(eps_t[:], EPS)
            ones_col = const.tile([P, 1], F32)
            nc.sync.dma_start(out=ones_col[:], in_=onesc[:])
            neg_t = const.tile([P, 1], F32)
            nc.vector.memset(neg_t[:], NEG)
            pb_t = const.tile([P, 1], F32)
            nc.sync.dma_start(out=pb_t[:], in_=pbias[:])
            tri_t = const.tile([P, P], F32)
            nc.sync.dma_start(out=tri_t[:], in_=tri[:])
            id_t = const.tile([P, P], F32R)
            nc.sync.dma_start(out=id_t[:], in_=ident[:].bitcast(F32R))
            std_all = const.tile([P, 16], F32)
            rstd_all = const.tile([P, 16], F32)
            rden = const.tile([P, NS], F32)
            memT_t = [const.tile([P, T_MEM], F32R, tag=f"memT{j}", name=f"memT{j}") for j in range(NJ)]
            for j in range(NJ):
                nc.sync.dma_start(out=memT_t[j][:],
                                  in_=memT[j * P:(j + 1) * P, :].bitcast(F32R))

            qT = [proj.tile([P, S_OWN], F32R, tag=f"qT{m}", name=f"qT{m}") for m in range(NJ)]
            kT = [proj.tile([P, S_OWN + T_PREF + T_MEM], F32R, tag=f"kT{m}", name=f"kT{m}")
                  for m in range(NJ)]

            # ================= phase A: norm, transpose, gate, projections
            with tc.tile_pool(name="aphase", bufs=1) as ap:
                normedT = [ap.tile([P, 2048], F32R, tag=f"nT{j}", name=f"nT{j}") for j in range(NJ)]

                # ---- A1: stats + normed + PE transpose + v spill
                with tc.tile_pool(name="a1s", bufs=2) as ast, \
                     tc.tile_pool(name="a1ps", bufs=4, space="PSUM") as aps:
                    w_bc_t = ast.tile([P, H], F32, bufs=1)
                    nc.sync.dma_start(out=w_bc_t[:], in_=w_bc[:])
                    sq = ast.tile([P, H], F32, bufs=1)  # shared Square scratch
                    for part in range(2):          # 0 = own, 1 = prefix
                        src = x_own if part == 0 else x_pref
                        for i in range(NS):
                            idx = part * NS + i
                            xt = ast.tile([P, H], F32, tag="xt")
                            nc.sync.dma_start(out=xt[:],
                                              in_=src[i * P:(i + 1) * P, :])
                            ss = ast.tile([P, 1], F32, tag="ss")
                            nc.scalar.activation(sq[:], xt[:], AFT.Square,
                                                 accum_out=ss[:])
                            nc.scalar.activation(std_all[:, idx:idx + 1], ss[:],
                                                 AFT.Sqrt, scale=1.0 / H,
                                                 bias=eps_t[:])
                            nc.vector.reciprocal(rstd_all[:, idx:idx + 1],
                                                 std_all[:, idx:idx + 1])
                            nrm = ast.tile([P, H], F32R, tag="nrm")
                            nc.scalar.activation(nrm[:], xt[:], AFT.Copy,
                                                 scale=rstd_all[:, idx:idx + 1])
                            # transpose 128x128 subtiles into normedT columns
                            for j in range(NJ):
                                tp = aps.tile([P, P], F32R, tag="tp")
                                nc.tensor.transpose(
                                    tp[:], nrm[:, j * P:(j + 1) * P], id_t[:])
                                nc.vector.tensor_copy(
                                    normedT[j][:, idx * P:(idx + 1) * P], tp[:])
                            # v = normed * norm_w (in place) -> spill
                            nc.vector.tensor_mul(nrm[:], nrm[:].bitcast(F32),
                                                 w_bc_t[:])
                            nc.sync.dma_start(
                                out=v_dram[idx * P:(idx + 1) * P, :],
                                in_=nrm[:].bitcast(F32))

                # ---- A2: gate -> alpha spill (gate = (normed @ gwT) * std)
                with tc.tile_pool(name="a2s", bufs=2) as gst, \
                     tc.tile_pool(name="a2ps", bufs=1, space="PSUM") as gps:
                    b_bc_t = gst.tile([P, H], F32, bufs=1)
                    nc.sync.dma_start(out=b_bc_t[:], in_=b_bc[:])
                    for oc in range(2):
                        pg = [gps.tile([P, 512], F32, tag=f"pg{si}",
                                       name=f"pg{si}") for si in range(NS)]
                        for j in range(NJ):
                            gwj = gst.tile([P, 512], F32R, tag="gwj")
                            nc.sync.dma_start(
                                out=gwj[:],
                                in_=gwT[j * P:(j + 1) * P,
                                        oc * 512:(oc + 1) * 512].bitcast(F32R))
                            for si in range(NS):
                                nc.tensor.matmul(
                                    pg[si][:],
                                    normedT[j][:, si * P:(si + 1) * P],
                                    gwj[:],
                                    start=(j == 0), stop=(j == NJ - 1))
                        for si in range(NS):
                            gl = gst.tile([P, 512], F32, tag="gl")
                            nc.scalar.activation(gl[:], pg[si][:], AFT.Copy,
                                                 scale=std_all[:, si:si + 1])
                            nc.vector.tensor_add(
                                gl[:], gl[:], b_bc_t[:, oc * 512:(oc + 1) * 512])
                            al = gst.tile([P, 512], F32, tag="al")
                            nc.scalar.activation(al[:], gl[:], AFT.Sigmoid)
                            nc.sync.dma_start(
                                out=a_dram[si * P:(si + 1) * P,
                                           oc * 512:(oc + 1) * 512],
                                in_=al[:])

                # ---- A3: qT / kT projections
                with tc.tile_pool(name="wstrip", bufs=2) as wsp, \
                     tc.tile_pool(name="a3ps", bufs=2, space="PSUM") as aps3:
                    for m in range(NJ):
                        qs = wsp.tile([P, H], F32R, tag="qs")
                        ks = wsp.tile([P, H], F32R, tag="ks")
                        for j in range(NJ):
                            nc.sync.dma_start(
                                out=qs[:, j * P:(j + 1) * P],
                                in_=qwT[j * P:(j + 1) * P,
                                        m * P:(m + 1) * P].bitcast(F32R))
                            nc.sync.dma_start(
                                out=ks[:, j * P:(j + 1) * P],
                                in_=kwT[j * P:(j + 1) * P,
                                        m * P:(m + 1) * P].bitcast(F32R))
                        # qT_m over own cols
                        for sc in range(2):
                            pq = aps3.tile([P, 512], F32, tag="pq")
                            for j in range(NJ):
                                nc.tensor.matmul(
                                    pq[:], qs[:, j * P:(j + 1) * P],
                                    normedT[j][:, sc * 512:(sc + 1) * 512],
                                    start=(j == 0), stop=(j == NJ - 1))
                            nc.vector.tensor_copy(
                                qT[m][:, sc * 512:(sc + 1) * 512], pq[:])
                        # kT_m over own+prefix cols
                        for sc in range(4):
                            pk = aps3.tile([P, 512], F32, tag="pq")
                            for j in range(NJ):
                                nc.tensor.matmul(
                                    pk[:], ks[:, j * P:(j + 1) * P],
                                    normedT[j][:, sc * 512:(sc + 1) * 512],
                                    start=(j == 0), stop=(j == NJ - 1))
                            nc.vector.tensor_copy(
                                kT[m][:, sc * 512:(sc + 1) * 512], pk[:])
                        # kT_m over memory cols
                        pkm = aps3.tile([P, T_MEM], F32, tag="pkm")
                        for j in range(NJ):
                            nc.tensor.matmul(pkm[:], ks[:, j * P:(j + 1) * P],
                                             memT_t[j][:],
                                             start=(j == 0), stop=(j == NJ - 1))
                        nc.vector.tensor_copy(kT[m][:, 2048:2048 + T_MEM], pkm[:])

            # ================= phase B: attention
            with tc.tile_pool(name="bres", bufs=1) as bres, \
                 tc.tile_pool(name="bstream", bufs=2) as bst, \
                 tc.tile_pool(name="bexp", bufs=3) as bexp, \
                 tc.tile_pool(name="bps", bufs=2, space="PSUM") as bps, \
                 tc.tile_pool(name="bph", bufs=1, space="PSUM") as bph:
                vpref = [bres.tile([P, H], F32R, tag=f"vp{t}", name=f"vp{t}") for t in range(8)]
                for t in range(8):
                    nc.sync.dma_start(
                        out=vpref[t][:],
                        in_=v_dram[S_OWN + t * P:S_OWN + (t + 1) * P, :]
                            .bitcast(F32R))
                vmem = bres.tile([T_MEM, H], F32R)
                nc.sync.dma_start(out=vmem[:], in_=mem[:].bitcast(F32R))

                NHG = 4  # half-groups of 2 stripes (256 query cols)
                for hg in range(NHG):
                    s0 = hg * 256
                    # tau blocks: (kind, index): own 0..2hg+1, prefix 0..7, mem
                    taus = ([("own", t) for t in range(2 * hg + 2)]
                            + [("pref", t) for t in range(8)]
                            + [("mem", 0)])
                    ph = {(sl, hc): bph.tile([P, 512], F32, tag=f"ph{sl}{hc}", name=f"ph{sl}{hc}")
                          for sl in range(2) for hc in range(2)}
                    pd = [bph.tile([P, 1], F32, tag=f"pd{sl}", name=f"pd{sl}")
                          for sl in range(2)]
                    for ti, (kind, t) in enumerate(taus):
                        first, last = ti == 0, ti == len(taus) - 1
                        rows = T_MEM if kind == "mem" else P
                        # scores^T [rows, 256]
                        ps = bps.tile([P, 256], F32, tag="ps")
                        if kind == "own":
                            koff = t * P
                        elif kind == "pref":
                            koff = S_OWN + t * P
                        else:
                            koff = 2048
                        for m in range(NJ):
                            nc.tensor.matmul(
                                ps[:rows, :], kT[m][:, koff:koff + rows],
                                qT[m][:, s0:s0 + 256],
                                start=(m == 0), stop=(m == NJ - 1))
                        # exp with fused scale (+mask / prefix bias)
                        et = bexp.tile([P, 256], F32R, tag="et")
                        if kind == "own":
                            sl_d = t - 2 * hg
                            if sl_d == 0:
                                nc.vector.tensor_add(ps[:, 0:P], ps[:, 0:P],
                                                     tri_t[:])
                                nc.scalar.activation(et[:], ps[:], AFT.Exp,
                                                     scale=SCALE)
                            elif sl_d == 1:
                                nc.scalar.activation(et[:, 0:P], ps[:, 0:P],
                                                     AFT.Exp, scale=SCALE,
                                                     bias=neg_t[:])
                                nc.vector.tensor_add(ps[:, P:256], ps[:, P:256],
                                                     tri_t[:])
                                nc.scalar.activation(et[:, P:256], ps[:, P:256],
                                                     AFT.Exp, scale=SCALE)
                            else:
                                nc.scalar.activation(et[:], ps[:], AFT.Exp,
                                                     scale=SCALE)
                        elif kind == "pref":
                            nc.scalar.activation(et[:], ps[:], AFT.Exp,
                                                 scale=SCALE, bias=pb_t[:])
                        else:
                            nc.scalar.activation(et[:rows, :], ps[:rows, :],
                                                 AFT.Exp, scale=SCALE)
                        # v tile
                        if kind == "own":
                            vt = bst.tile([P, H], F32R, tag="vb", bufs=3)
                            nc.sync.dma_start(
                                out=vt[:],
                                in_=v_dram[t * P:(t + 1) * P, :].bitcast(F32R))
                        elif kind == "pref":
                            vt = vpref[t]
                        else:
                            vt = vmem
                        # h~ accumulation + denominator (same stationary et)
                        for sl in range(2):
                            for hc in range(2):
                                nc.tensor.matmul(
                                    ph[(sl, hc)][:],
                                    et[:rows, sl * P:(sl + 1) * P],
                                    vt[:rows, hc * 512:(hc + 1) * 512],
                                    start=first, stop=last,
                                    skip_group_check=True)
                            nc.tensor.matmul(
                                pd[sl][:],
                                et[:rows, sl * P:(sl + 1) * P].bitcast(F32),
                                ones_col[:rows, :],
                                start=first, stop=last,
                                skip_group_check=True)
                    for sl in range(2):
                        sidx = 2 * hg + sl
                        nc.vector.reciprocal(rden[:, sidx:sidx + 1], pd[sl][:])
                    # evict h~, final combine
                    for sl in range(2):
                        sidx = 2 * hg + sl
                        hsb = bst.tile([P, H], F32, tag="hsb")
                        for hc in range(2):
                            nc.scalar.activation(
                                hsb[:, hc * 512:(hc + 1) * 512], ph[(sl, hc)][:],
                                AFT.Copy, scale=rden[:, sidx:sidx + 1])
                        xs = bst.tile([P, H], F32, tag="xs")
                        nc.sync.dma_start(out=xs[:],
                                          in_=x_own[sidx * P:(sidx + 1) * P, :])
                        als = bst.tile([P, H], F32, tag="als")
                        nc.sync.dma_start(out=als[:],
                                          in_=a_dram[sidx * P:(sidx + 1) * P, :])
                        nc.vector.tensor_sub(hsb[:], hsb[:], xs[:])
                        nc.vector.tensor_mul(hsb[:], hsb[:], als[:])
                        nc.vector.tensor_add(hsb[:], hsb[:], xs[:])
                        nc.sync.dma_start(out=out[sidx * P:(sidx + 1) * P, :],
                                          in_=hsb[:])
                if DEBUG:
                    nc.sync.dma_start(out=dbg_rden[:], in_=rden[:])

    import os
    if os.environ.get("NO_WAIT_SPLIT") != "1":
        split_multi_waits(nc, limit=1, dma_limit=1)
    return nc


_NC_CACHE = None
_LAST_IN_MAPS = None


def _get_nc():
    global _NC_CACHE
    if _NC_CACHE is None:
        _NC_CACHE = build_nc()
    return _NC_CACHE


def prepare_in_maps(hidden_states, memory_state, q_w, k_w, norm_w, gate_w,
                    gate_b):
    hidden_states = np.asarray(hidden_states, dtype=np.float32)
    memory_state = np.asarray(memory_state, dtype=np.float32)
    q_w = np.asarray(q_w, dtype=np.float32)
    k_w = np.asarray(k_w, dtype=np.float32)
    norm_w = np.asarray(norm_w, dtype=np.float32)
    gate_w = np.asarray(gate_w, dtype=np.float32)
    gate_b = np.asarray(gate_b, dtype=np.float32)

    qwT = np.ascontiguousarray((q_w * norm_w[None, :]).T)
    kwT = np.ascontiguousarray((k_w * norm_w[None, :]).T)
    gwT = np.ascontiguousarray(gate_w.T)
    w_bc = np.ascontiguousarray(np.broadcast_to(norm_w, (P, H)))
    b_bc = np.ascontiguousarray(np.broadcast_to(gate_b, (P, H)))
    tri = np.where(np.arange(P)[None, :] >= np.arange(P)[:, None],
                   np.float32(0.0), np.float32(NEG)).astype(np.float32)
    ident = np.eye(P, dtype=np.float32)
    zeros_pref = np.zeros((T_PREF, H), dtype=np.float32)

    in_maps = []
    for c in range(N_CORES):
        b, h = divmod(c, 2)
        x_own = np.ascontiguousarray(hidden_states[b, h * S_OWN:(h + 1) * S_OWN])
        x_pref = (np.ascontiguousarray(hidden_states[b, :T_PREF]) if h == 1
                  else zeros_pref)
        memb = np.ascontiguousarray(memory_state[b])
        in_maps.append({
            "x_own": x_own,
            "x_pref": x_pref,
            "mem": memb,
            "memT": np.ascontiguousarray(memb.T),
            "qwT": qwT, "kwT": kwT, "gwT": gwT,
            "w_bc": w_bc, "b_bc": b_bc,
            "pbias": np.full((P, 1), 0.0 if h == 1 else NEG, np.float32),
            "onesc": np.ones((P, 1), np.float32),
            "tri": tri, "ident": ident,
        })
    return in_maps


def kernel(**inputs):
    in_maps = prepare_in_maps(**inputs)
    global _LAST_IN_MAPS
    _LAST_IN_MAPS = in_maps
    nc = _get_nc()
    res = run_bass_kernel_spmd(nc, in_maps, list(range(N_CORES)))
    out = np.empty((B_FULL, S_FULL, H), dtype=np.float32)
    for c in range(N_CORES):
        b, h = divmod(c, 2)
        out[b, h * S_OWN:(h + 1) * S_OWN] = res.results[c]["out"]
    return out



# revision 3
# speedup vs baseline: 1.6557x; 1.6557x over previous
"""Trainium2 Bass kernel for nn_MemResProjections — v3.

v2 -> v3 changes (from v2 trace: ACT 60% busy = bottleneck, PE 85us gaps,
first matmul at t=35us, fp8 projections/scores fail accuracy):
  * G-trick: logits = n_s^T G n_t with G = diag(nw) q_w^T k_w diag(nw)
    precomputed on host -> the whole q projection disappears; scores
    consume normed^T directly (rhs) against kTg = G @ n~ (lhsT).
  * dtypes: bf16 projections + scores (accuracy), fp8 DoubleRow h~ path
    with a first-order residual on v (v8 + vr8), bf16 memory-v path
    (fp8 vmem alone cost 1.4e-2 rel err).  Predicted rel err ~8e-3.
  * norm_w folded into the output combine (v holds raw normed rows).
  * ACT batching: paired exp tiles [128,512], single-op gate eviction
    chain per stripe ([128,1024] psum), single-op hsb eviction.
  * PE warmup matmuls + x-stripe DMAs issued before weight DMAs.

Sharding (unchanged from v2): 8 cores = (batch, parity); core owns the
odd/even 128-stripes of its batch, all tensors in own-first coordinates;
causal envelope per 256-query slot is position ranges {0..2k+1} u
{8..9+2k} + mem on every core (uniform SPMD), fringe masks are per-core
DRAM data.
"""
import numpy as np
import ml_dtypes

import concourse.bass as bass
import concourse.mybir as mybir
import concourse.tile as tile
from concourse.bass_utils import run_bass_kernel_spmd

F32 = mybir.dt.float32
BF16 = mybir.dt.bfloat16
FP8 = mybir.dt.float8e4
FP16 = mybir.dt.float16
AFT = mybir.ActivationFunctionType
DR = mybir.MatmulPerfMode.DoubleRow

NP_BF16 = ml_dtypes.bfloat16
NP_FP8 = ml_dtypes.float8_e4m3

P = 128
H = 1024
NJ = 8
NS = 16
T_MEM = 64
TKV = 2048 + T_MEM
SCALE = 1.0 / 32.0
EXP_SHIFT = -2.0
EPS = 1e-6
NEG = -1.0e30

N_CORES = 8
B_FULL, S_FULL = 4, 2048

ENGINE_ATTR = {
    mybir.EngineType.PE: "tensor",
    mybir.EngineType.Activation: "scalar",
    mybir.EngineType.DVE: "vector",
    mybir.EngineType.Pool: "gpsimd",
    mybir.EngineType.SP: "sync",
}
DMA_OPS = ("InstDMACopy", "InstDMATranspose", "InstTensorLoad", "InstTensorSave",
           "InstCollectiveCompute")


def split_multi_waits(nc, limit=1, dma_limit=None):
    """This walrus build rejects engine instructions carrying more than one
    sem wait; hoist extras onto same-engine NOPs inserted just before."""
    n_split = 0
    for f in nc.m.functions:
        for blk in f.blocks:
            il = blk.instructions
            i = 0
            while i < len(il):
                ins = il[i]
                is_dma = type(ins).__name__ in DMA_OPS
                lim = dma_limit if is_dma else limit
                si = ins.sync_info
                waits = list(si.on_wait) if si is not None and si.on_wait else []
                if lim is not None and len(waits) > lim:
                    keep, extra = waits[:lim], waits[lim:]
                    si.on_wait.clear()
                    for w in keep:
                        si.on_wait.append(w)
                    eng = getattr(nc, ENGINE_ATTR[ins.engine])
                    for w in extra:
                        nop = eng.nop(nofuse=True, hint="wait_split")
                        nop.wait_op(bass.SemaphoreHandle(w.ant_name, w.id),
                                    w.wait_value, "sem-ge")
                        popped = nc.cur_bb.bb.instructions.pop()
                        assert popped.name == nop.ins.name
                        il.insert(i, nop.ins)
                        i += 1
                        n_split += 1
                i += 1
    return n_split


def build_nc():
    nc = bass.Bass()
    dp = lambda n, shp, dt=F32: nc.declare_dram_parameter(n, shp, dt,
                                                          isOutput=False)
    x_full = dp("x_full", [2048, H])
    Gw16 = dp("Gw16", [P, NJ, H], BF16)      # G.T as [p, j, o]
    gw16 = dp("gw16", [P, NJ, H], BF16)      # gate_w.T as [p, j, o]
    memx16 = dp("memx16", [P, NJ, T_MEM], BF16)
    vmem16 = dp("vmem16", [T_MEM, H], BF16)
    masks = dp("masks", [P, 4 * 256])
    w_bc32 = dp("w_bc32", [P, H])            # norm_w broadcast (output fold)
    b_bc = dp("b_bc", [P, H])
    id16 = dp("id16", [P, P], BF16)
    ones8 = dp("ones8", [P, 2, 1], FP8)
    out = nc.declare_dram_parameter("out", [1024, H], F32, isOutput=True)

    with tile.TileContext(nc) as tc:
        from contextlib import ExitStack
        with ExitStack() as ctx:
            const = ctx.enter_context(tc.tile_pool(name="const", bufs=1))
            res = ctx.enter_context(tc.tile_pool(name="res", bufs=1))

            # --- x stripes of the first group queue ahead of weights
            xt_first = []
            with tc.tile_pool(name="xfirst", bufs=1) as xf:
                for i in range(2):
                    xt = xf.tile([P, H], F32, tag=f"x{i}", name=f"x{i}")
                    nc.sync.dma_start(out=xt[:],
                                      in_=x_full[i * P:(i + 1) * P, :])
                    xt_first.append(xt)

                id_t = const.tile([P, P], BF16)
                nc.sync.dma_start(out=id_t[:], in_=id16[:])
                mask_t = const.tile([P, 4 * 256], F32)
                nc.sync.dma_start(out=mask_t[:], in_=masks[:])
                w_bc_t = const.tile([P, H], F32)
                nc.sync.dma_start(out=w_bc_t[:], in_=w_bc32[:])
                b_bc_t = const.tile([P, H], F32)
                nc.sync.dma_start(out=b_bc_t[:], in_=b_bc[:])
                ones_t = const.tile([P, 2, 1], FP8)
                nc.sync.dma_start(out=ones_t[:], in_=ones8[:])
                ones16_t = const.tile([T_MEM, 1], BF16)
                nc.vector.memset(ones16_t[:], 1.0)
                std_all = const.tile([P, NS], F32)
                rstd_all = const.tile([P, NS], F32)
                rden = const.tile([P, NJ], F32)
                eps_t = const.tile([P, 1], F32)
                nc.vector.memset(eps_t[:], EPS)
                shift_t = const.tile([P, 1], F32)
                nc.vector.memset(shift_t[:], EXP_SHIFT)
                wup = const.tile([P, P], F32)

                vmem_t = res.tile([T_MEM, H], BF16)
                nc.sync.dma_start(out=vmem_t[:], in_=vmem16[:])
                nT = res.tile([P, NJ, TKV], BF16)
                nc.sync.dma_start(out=nT[:, :, 2048:TKV], in_=memx16[:])
                v8 = res.tile([P, NS, H], FP8)
                vr8 = res.tile([P, NS, H], FP8)
                kTg = res.tile([P, NJ, TKV], BF16)
                alpha = res.tile([P, NJ, H], FP16)

                gw_t = res.tile([P, NJ, H], BF16)
                nc.sync.dma_start(out=gw_t[:], in_=gw16[:])
                Gw_t = res.tile([P, NJ, H], BF16)
                nc.sync.dma_start(out=Gw_t[:], in_=Gw16[:])

                # ============ phase A: pipelined norm/transpose/projections
                with tc.tile_pool(name="aw", bufs=1) as aw, \
                     tc.tile_pool(name="aps", bufs=2, space="PSUM") as aps, \
                     tc.tile_pool(name="apj", bufs=2, space="PSUM") as apj, \
                     tc.tile_pool(name="apg", bufs=2, space="PSUM") as apg:
                    # PE warmup: keep HAM busy while stripe 0 stats run
                    tpw = apj.tile([P, 512], F32, tag="pj")
                    for w in range(40):
                        nc.tensor.matmul(tpw[:, 0:P], id_t[:], id_t[:],
                                         start=True, stop=True)
                    nc.vector.tensor_copy(wup[:], tpw[:, 0:P])

                    for g in range(4):
                        for i in range(4 * g, 4 * g + 4):
                            if i < 2:
                                xt = xt_first[i]
                            else:
                                xt = aw.tile([P, H], F32, tag="xt", bufs=3)
                                nc.sync.dma_start(
                                    out=xt[:], in_=x_full[i * P:(i + 1) * P, :])
                            sq = aw.tile([P, H], BF16, tag="sq", bufs=2)
                            ss = aw.tile([P, 1], F32, tag="ss", bufs=2)
                            nc.scalar.activation(sq[:], xt[:], AFT.Square,
                                                 accum_out=ss[:])
                            nc.scalar.activation(std_all[:, i:i + 1], ss[:],
                                                 AFT.Sqrt, scale=1.0 / H,
                                                 bias=eps_t[:])
                            nc.vector.reciprocal(rstd_all[:, i:i + 1],
                                                 std_all[:, i:i + 1])
                            nrm = aw.tile([P, H], BF16, tag="nrm", bufs=2)
                            nc.scalar.activation(nrm[:], xt[:], AFT.Copy,
                                                 scale=rstd_all[:, i:i + 1])
                            nc.gpsimd.tensor_copy(v8[:, i, :], nrm[:])
                            nc.vector.tensor_sub(vr8[:, i, :], nrm[:],
                                                 v8[:, i, :])
                            for half in range(2):
                                tp = aps.tile([P, 512], F32, tag="tp")
                                for jj in range(4):
                                    j = half * 4 + jj
                                    nc.tensor.matmul(
                                        tp[:, jj * P:(jj + 1) * P],
                                        nrm[:, j * P:(j + 1) * P], id_t[:],
                                        start=True, stop=True)
                                j0 = half * 4
                                nc.vector.tensor_copy(
                                    nT[:, j0:j0 + 4, i * P:(i + 1) * P], tp[:])

                        c0, c1 = g * 512, (g + 1) * 512
                        # kTg for this column group (bf16)
                        for m in range(NJ):
                            pk = apj.tile([P, 512], F32, tag="pj")
                            for j in range(NJ):
                                nc.tensor.matmul(
                                    pk[:], Gw_t[:, j, m * P:(m + 1) * P],
                                    nT[:, j, c0:c1],
                                    start=(j == 0), stop=(j == NJ - 1))
                            nc.scalar.activation(kTg[:, m, c0:c1], pk[:],
                                                 AFT.Copy)
                        if g < 2:
                            # gate for the 4 own stripes of this group
                            for si in range(4 * g, 4 * g + 4):
                                pg = apg.tile([P, 1024], F32, tag="pg")
                                for oc in range(2):
                                    for j in range(NJ):
                                        nc.tensor.matmul(
                                            pg[:, oc * 512:(oc + 1) * 512],
                                            nT[:, j, si * P:(si + 1) * P],
                                            gw_t[:, j, oc * 512:(oc + 1) * 512],
                                            start=(j == 0), stop=(j == NJ - 1))
                                glf = aw.tile([P, H], F32, tag="glf", bufs=2)
                                nc.scalar.activation(glf[:], pg[:], AFT.Copy,
                                                     scale=std_all[:, si:si + 1])
                                nc.vector.tensor_add(glf[:], glf[:], b_bc_t[:])
                                nc.scalar.activation(alpha[:, si, :], glf[:],
                                                     AFT.Sigmoid)
                    # memory kTg columns
                    for m in range(NJ):
                        pkm = apj.tile([P, T_MEM], F32, tag="pj")
                        for j in range(NJ):
                            nc.tensor.matmul(
                                pkm[:], Gw_t[:, j, m * P:(m + 1) * P],
                                nT[:, j, 2048:TKV],
                                start=(j == 0), stop=(j == NJ - 1))
                        nc.scalar.activation(kTg[:, m, 2048:TKV], pkm[:],
                                             AFT.Copy)

            # ============ phase B: attention
            with tc.tile_pool(name="bw", bufs=1) as bw, \
                 tc.tile_pool(name="bexp", bufs=3) as bexp, \
                 tc.tile_pool(name="bps", bufs=2, space="PSUM") as bps, \
                 tc.tile_pool(name="bph", bufs=1, space="PSUM") as bph:
                for k in range(4):
                    q0 = k * 256
                    ph = [bph.tile([P, 1024], F32, tag=f"ph{sl}",
                                   name=f"ph{sl}") for sl in range(2)]
                    pd = [bph.tile([P, 1], F32, tag=f"pd{sl}", name=f"pd{sl}")
                          for sl in range(2)]
                    # jobs: own pairs, other pairs, then memory
                    jobs = ([("own", 2 * pi) for pi in range(k + 1)]
                            + [("oth", 8 + 2 * pi) for pi in range(k + 1)]
                            + [("mem", 16)])
                    ets = []

                    def emit_scores(ji):
                        kind, pos = jobs[ji]
                        if kind == "mem":
                            et = bexp.tile([T_MEM, 256], BF16, tag="etm")
                            ps = bps.tile([P, 512], F32, tag="ps")
                            for j in range(NJ):
                                nc.tensor.matmul(
                                    ps[:T_MEM, 0:256],
                                    kTg[:, j, 2048:TKV],
                                    nT[:, j, q0:q0 + 256],
                                    start=(j == 0), stop=(j == NJ - 1))
                            nc.scalar.activation(et[:], ps[:T_MEM, 0:256],
                                                 AFT.Exp, scale=SCALE,
                                                 bias=shift_t[:T_MEM, :])
                            ets.append(et)
                            return
                        et = bexp.tile([P, 2, 256], FP8, tag="et")
                        ps = bps.tile([P, 512], F32, tag="ps")
                        for ko in range(2):
                            t = pos + ko
                            for j in range(NJ):
                                nc.tensor.matmul(
                                    ps[:, ko * 256:(ko + 1) * 256],
                                    kTg[:, j, t * P:(t + 1) * P],
                                    nT[:, j, q0:q0 + 256],
                                    start=(j == 0), stop=(j == NJ - 1))
                        if kind == "own" and pos == 2 * k:
                            nc.vector.tensor_add(ps[:], ps[:],
                                                 mask_t[:, 0:512])
                        elif kind == "oth" and pos == 8 + 2 * k:
                            nc.vector.tensor_add(ps[:], ps[:],
                                                 mask_t[:, 512:1024])
                        nc.scalar.activation(et[:, 0:2, :], ps[:], AFT.Exp,
                                             scale=SCALE, bias=shift_t[:])
                        ets.append(et)

                    def emit_hacc(ji):
                        kind, pos = jobs[ji]
                        first = ji == 0
                        last = ji == len(jobs) - 1
                        et = ets[ji]
                        for sl in range(2):
                            if kind == "mem":
                                lt = et[:, sl * P:(sl + 1) * P]
                                for hc in range(2):
                                    nc.tensor.matmul(
                                        ph[sl][:, hc * 512:(hc + 1) * 512], lt,
                                        vmem_t[:, hc * 512:(hc + 1) * 512],
                                        start=first, stop=last,
                                        skip_group_check=True)
                                nc.tensor.matmul(
                                    pd[sl][:], lt, ones16_t[:],
                                    start=first, stop=last,
                                    skip_group_check=True)
                            else:
                                lt = et[:, 0:2, sl * P:(sl + 1) * P]
                                for hc in range(2):
                                    nc.tensor.matmul(
                                        ph[sl][:, hc * 512:(hc + 1) * 512], lt,
                                        v8[:, pos:pos + 2,
                                           hc * 512:(hc + 1) * 512],
                                        start=first, stop=False,
                                        perf_mode=DR, skip_group_check=True)
                                    nc.tensor.matmul(
                                        ph[sl][:, hc * 512:(hc + 1) * 512], lt,
                                        vr8[:, pos:pos + 2,
                                            hc * 512:(hc + 1) * 512],
                                        start=False, stop=False,
                                        perf_mode=DR, skip_group_check=True)
                                nc.tensor.matmul(
                                    pd[sl][:], lt, ones_t[:, 0:2, :],
                                    start=first, stop=False,
                                    perf_mode=DR, skip_group_check=True)

                    for ji in range(len(jobs)):
                        emit_scores(ji)
                        if ji >= 1:
                            emit_hacc(ji - 1)
                    emit_hacc(len(jobs) - 1)

                    for sl in range(2):
                        sidx = 2 * k + sl
                        nc.vector.reciprocal(rden[:, sidx:sidx + 1], pd[sl][:])
                        hsb = bw.tile([P, H], F32, tag="hsb", bufs=2)
                        nc.scalar.activation(hsb[:], ph[sl][:], AFT.Copy,
                                             scale=rden[:, sidx:sidx + 1])
                        xs = bw.tile([P, H], F32, tag="xs", bufs=2)
                        nc.sync.dma_start(
                            out=xs[:], in_=x_full[sidx * P:(sidx + 1) * P, :])
                        a32 = bw.tile([P, H], F32, tag="a32", bufs=2)
                        nc.vector.tensor_copy(a32[:], alpha[:, sidx, :])
                        nc.vector.tensor_mul(hsb[:], hsb[:], w_bc_t[:])
                        nc.vector.tensor_sub(hsb[:], hsb[:], xs[:])
                        nc.vector.tensor_mul(hsb[:], hsb[:], a32[:])
                        nc.gpsimd.tensor_add(hsb[:], hsb[:], xs[:])
                        nc.sync.dma_start(out=out[sidx * P:(sidx + 1) * P, :],
                                          in_=hsb[:])

    import os
    if os.environ.get("NO_WAIT_SPLIT") != "1":
        split_multi_waits(nc, limit=1, dma_limit=1)
    return nc


_NC_CACHE = None
_LAST_IN_MAPS = None


def _get_nc():
    global _NC_CACHE
    if _NC_CACHE is None:
        _NC_CACHE = build_nc()
    return _NC_CACHE


def _mk_masks(h):
    tri = np.where(np.arange(P)[None, :] >= np.arange(P)[:, None],
                   np.float32(0.0), np.float32(NEG)).astype(np.float32)
    Z = np.zeros((P, P), np.float32)
    NB = np.full((P, P), NEG, np.float32)
    m0 = np.concatenate([tri, Z], axis=1)
    m1 = np.concatenate([NB, tri], axis=1)
    if h == 0:
        m2 = np.concatenate([NB, Z], axis=1)
        m3 = np.concatenate([NB, NB], axis=1)
    else:
        m2 = np.concatenate([Z, Z], axis=1)
        m3 = np.concatenate([NB, Z], axis=1)
    return np.concatenate([m0, m1, m2, m3], axis=1)


def prepare_in_maps(hidden_states, memory_state, q_w, k_w, norm_w, gate_w,
                    gate_b):
    hidden_states = np.asarray(hidden_states, dtype=np.float32)
    memory_state = np.asarray(memory_state, dtype=np.float32)
    q_w = np.asarray(q_w, dtype=np.float32)
    k_w = np.asarray(k_w, dtype=np.float32)
    norm_w = np.asarray(norm_w, dtype=np.float32)
    gate_w = np.asarray(gate_w, dtype=np.float32)
    gate_b = np.asarray(gate_b, dtype=np.float32)

    def wrearrange(wT):   # [h, o] -> [p, j, o]
        return np.ascontiguousarray(wT.reshape(NJ, P, H).transpose(1, 0, 2))

    G = (q_w * norm_w[None, :]).T @ (k_w * norm_w[None, :])   # [h, h']
    Gw16 = wrearrange(np.ascontiguousarray(G.T)).astype(NP_BF16)
    gw16 = wrearrange(np.ascontiguousarray(gate_w.T)).astype(NP_BF16)
    w_bc32 = np.ascontiguousarray(np.broadcast_to(norm_w, (P, H)))
    b_bc = np.ascontiguousarray(np.broadcast_to(gate_b, (P, H)))
    id16 = np.eye(P, dtype=np.float32).astype(NP_BF16)
    ones8 = np.ones((P, 2, 1), np.float32).astype(NP_FP8)
    safe_nw = np.where(norm_w == 0.0, 1.0, norm_w)

    in_maps = []
    for c in range(N_CORES):
        b, h = divmod(c, 2)
        xs = hidden_states[b].reshape(NS, P, H)
        x_full = np.ascontiguousarray(
            np.concatenate([xs[h::2], xs[1 - h::2]], axis=0).reshape(2048, H))
        memb = memory_state[b]
        memx16 = np.ascontiguousarray(
            (memb / safe_nw[None, :]).T.reshape(NJ, P, T_MEM)
            .transpose(1, 0, 2)).astype(NP_BF16)
        in_maps.append({
            "x_full": x_full,
            "Gw16": Gw16, "gw16": gw16,
            "memx16": memx16,
            "vmem16": np.ascontiguousarray(memb).astype(NP_BF16),
            "masks": _mk_masks(h),
            "w_bc32": w_bc32, "b_bc": b_bc,
            "id16": id16, "ones8": ones8,
        })
    return in_maps


def kernel(**inputs):
    in_maps = prepare_in_maps(**inputs)
    global _LAST_IN_MAPS
    _LAST_IN_MAPS = in_maps
    nc = _get_nc()
    res = run_bass_kernel_spmd(nc, in_maps, list(range(N_CORES)))
    out = np.empty((B_FULL, S_FULL, H), dtype=np.float32)
    for c in range(N_CORES):
        b, h = divmod(c, 2)
        o = res.results[c]["out"].reshape(NJ, P, H)
        for i in range(NJ):
            out[b, (2 * i + h) * P:(2 * i + h + 1) * P] = o[i]
    return out


# revision 4
# speedup vs baseline: 1.9797x; 1.1957x over previous
"""Trainium2 Bass kernel for nn_MemResProjections — v3.

v2 -> v3 changes (from v2 trace: ACT 60% busy = bottleneck, PE 85us gaps,
first matmul at t=35us, fp8 projections/scores fail accuracy):
  * G-trick: logits = n_s^T G n_t with G = diag(nw) q_w^T k_w diag(nw)
    precomputed on host -> the whole q projection disappears; scores
    consume normed^T directly (rhs) against kTg = G @ n~ (lhsT).
  * dtypes: bf16 projections + scores (accuracy), fp8 DoubleRow h~ path
    with a first-order residual on v (v8 + vr8), bf16 memory-v path
    (fp8 vmem alone cost 1.4e-2 rel err).  Predicted rel err ~8e-3.
  * norm_w folded into the output combine (v holds raw normed rows).
  * ACT batching: paired exp tiles [128,512], single-op gate eviction
    chain per stripe ([128,1024] psum), single-op hsb eviction.
  * PE warmup matmuls + x-stripe DMAs issued before weight DMAs.

Sharding (unchanged from v2): 8 cores = (batch, parity); core owns the
odd/even 128-stripes of its batch, all tensors in own-first coordinates;
causal envelope per 256-query slot is position ranges {0..2k+1} u
{8..9+2k} + mem on every core (uniform SPMD), fringe masks are per-core
DRAM data.
"""
import numpy as np
import ml_dtypes

import concourse.bass as bass
import concourse.mybir as mybir
import concourse.tile as tile
from concourse.bass_utils import run_bass_kernel_spmd

F32 = mybir.dt.float32
BF16 = mybir.dt.bfloat16
FP8 = mybir.dt.float8e4
FP16 = mybir.dt.float16
AFT = mybir.ActivationFunctionType
DR = mybir.MatmulPerfMode.DoubleRow

NP_BF16 = ml_dtypes.bfloat16
NP_FP8 = ml_dtypes.float8_e4m3

P = 128
H = 1024
NJ = 8
NS = 16
T_MEM = 64
TKV = 2048 + T_MEM
SCALE = 1.0 / 32.0
EXP_SHIFT = -2.0
EPS = 1e-6
NEG = -1.0e30

N_CORES = 8
B_FULL, S_FULL = 4, 2048

ENGINE_ATTR = {
    mybir.EngineType.PE: "tensor",
    mybir.EngineType.Activation: "scalar",
    mybir.EngineType.DVE: "vector",
    mybir.EngineType.Pool: "gpsimd",
    mybir.EngineType.SP: "sync",
}
DMA_OPS = ("InstDMACopy", "InstDMATranspose", "InstTensorLoad", "InstTensorSave",
           "InstCollectiveCompute")


def split_multi_waits(nc, limit=1, dma_limit=None):
    """This walrus build rejects engine instructions carrying more than one
    sem wait; hoist extras onto same-engine NOPs inserted just before."""
    n_split = 0
    for f in nc.m.functions:
        for blk in f.blocks:
            il = blk.instructions
            i = 0
            while i < len(il):
                ins = il[i]
                is_dma = type(ins).__name__ in DMA_OPS
                lim = dma_limit if is_dma else limit
                si = ins.sync_info
                waits = list(si.on_wait) if si is not None and si.on_wait else []
                if lim is not None and len(waits) > lim:
                    keep, extra = waits[:lim], waits[lim:]
                    si.on_wait.clear()
                    for w in keep:
                        si.on_wait.append(w)
                    eng = getattr(nc, ENGINE_ATTR[ins.engine])
                    for w in extra:
                        nop = eng.nop(nofuse=True, hint="wait_split")
                        nop.wait_op(bass.SemaphoreHandle(w.ant_name, w.id),
                                    w.wait_value, "sem-ge")
                        popped = nc.cur_bb.bb.instructions.pop()
                        assert popped.name == nop.ins.name
                        il.insert(i, nop.ins)
                        i += 1
                        n_split += 1
                i += 1
    return n_split


def build_nc():
    nc = bass.Bass()
    dp = lambda n, shp, dt=F32: nc.declare_dram_parameter(n, shp, dt,
                                                          isOutput=False)
    x_full = dp("x_full", [2048, H])
    Gw16 = dp("Gw16", [P, NJ, H], BF16)      # G.T as [p, j, o]
    gw16 = dp("gw16", [P, NJ, H], BF16)      # gate_w.T as [p, j, o]
    memx16 = dp("memx16", [P, NJ, T_MEM], BF16)
    vmem16 = dp("vmem16", [T_MEM, H], BF16)
    masks = dp("masks", [P, 4 * 256])
    w_bc32 = dp("w_bc32", [P, H])            # norm_w broadcast (output fold)
    b_bc = dp("b_bc", [P, H])
    id16 = dp("id16", [P, P], BF16)
    ones8 = dp("ones8", [P, 2, 1], FP8)
    out = nc.declare_dram_parameter("out", [1024, H], F32, isOutput=True)

    with tile.TileContext(nc) as tc:
        from contextlib import ExitStack
        with ExitStack() as ctx:
            const = ctx.enter_context(tc.tile_pool(name="const", bufs=1))
            res = ctx.enter_context(tc.tile_pool(name="res", bufs=1))

            # --- x stripes of the first group queue ahead of weights
            xt_first = []
            with tc.tile_pool(name="xfirst", bufs=1) as xf:
                id_t = const.tile([P, P], BF16)
                nc.sync.dma_start(out=id_t[:], in_=id16[:])
                for i in range(2):
                    xt = xf.tile([P, H], F32, tag=f"x{i}", name=f"x{i}")
                    nc.sync.dma_start(out=xt[:],
                                      in_=x_full[i * P:(i + 1) * P, :])
                    xt_first.append(xt)
                mask_t = const.tile([P, 4 * 256], F32)
                nc.sync.dma_start(out=mask_t[:], in_=masks[:])
                w_bc_t = const.tile([P, H], F32)
                nc.sync.dma_start(out=w_bc_t[:], in_=w_bc32[:])
                b_bc_t = const.tile([P, H], F32)
                nc.sync.dma_start(out=b_bc_t[:], in_=b_bc[:])
                ones_t = const.tile([P, 2, 1], FP8)
                nc.sync.dma_start(out=ones_t[:], in_=ones8[:])
                ones16_t = const.tile([T_MEM, 1], BF16)
                nc.vector.memset(ones16_t[:], 1.0)
                std_all = const.tile([P, NS], F32)
                rstd_all = const.tile([P, NS], F32)
                rden = const.tile([P, NJ], F32)
                eps_t = const.tile([P, 1], F32)
                nc.vector.memset(eps_t[:], EPS)
                shift_t = const.tile([P, 1], F32)
                nc.vector.memset(shift_t[:], EXP_SHIFT)
                wup = const.tile([P, P], F32)

                vmem_t = res.tile([T_MEM, H], BF16)
                nc.sync.dma_start(out=vmem_t[:], in_=vmem16[:])
                nT = res.tile([P, NJ, TKV], BF16)
                nc.sync.dma_start(out=nT[:, :, 2048:TKV], in_=memx16[:])
                v8 = res.tile([P, NS, H], FP8)
                vr8 = res.tile([P, NS, H], FP8)
                kTg = res.tile([P, NJ, TKV], BF16)
                alpha = res.tile([P, NJ, H], FP16)

                gw_t = res.tile([P, NJ, H], BF16)
                nc.sync.dma_start(out=gw_t[:], in_=gw16[:])
                Gw_t = res.tile([P, NJ, H], BF16)
                nc.sync.dma_start(out=Gw_t[:], in_=Gw16[:])

                # ============ phase A: pipelined norm/transpose/projections
                with tc.tile_pool(name="aw", bufs=1) as aw, \
                     tc.tile_pool(name="aps", bufs=2, space="PSUM") as aps, \
                     tc.tile_pool(name="apj", bufs=2, space="PSUM") as apj, \
                     tc.tile_pool(name="apg", bufs=2, space="PSUM") as apg:
                    # PE warmup: keep HAM busy while stripe 0 stats run
                    tpw = apj.tile([P, 512], F32, tag="pj")
                    for w in range(40):
                        nc.tensor.matmul(tpw[:, 0:P], id_t[:], id_t[:],
                                         start=True, stop=True)
                    nc.vector.tensor_copy(wup[:], tpw[:, 0:P])

                    def emit_stripes(g):
                        for i in range(4 * g, 4 * g + 4):
                            if i < 2:
                                xt = xt_first[i]
                            else:
                                xt = aw.tile([P, H], F32, tag="xt", bufs=3)
                                nc.sync.dma_start(
                                    out=xt[:], in_=x_full[i * P:(i + 1) * P, :])
                            sq = aw.tile([P, H], BF16, tag="sq", bufs=2)
                            ss = aw.tile([P, 1], F32, tag="ss", bufs=2)
                            nc.scalar.activation(sq[:], xt[:], AFT.Square,
                                                 accum_out=ss[:])
                            nc.scalar.activation(std_all[:, i:i + 1], ss[:],
                                                 AFT.Sqrt, scale=1.0 / H,
                                                 bias=eps_t[:])
                            nc.vector.reciprocal(rstd_all[:, i:i + 1],
                                                 std_all[:, i:i + 1])
                            nrm = aw.tile([P, H], BF16, tag="nrm", bufs=2)
                            nc.scalar.activation(nrm[:], xt[:], AFT.Copy,
                                                 scale=rstd_all[:, i:i + 1])
                            nc.scalar.activation(v8[:, i, :], xt[:],
                                                 AFT.Copy,
                                                 scale=rstd_all[:, i:i + 1])
                            nc.vector.tensor_sub(vr8[:, i, :], nrm[:],
                                                 v8[:, i, :])
                            for half in range(2):
                                tp = aps.tile([P, 512], F32, tag="tp")
                                for jj in range(4):
                                    j = half * 4 + jj
                                    nc.tensor.matmul(
                                        tp[:, jj * P:(jj + 1) * P],
                                        nrm[:, j * P:(j + 1) * P], id_t[:],
                                        start=True, stop=True)
                                j0 = half * 4
                                nc.vector.tensor_copy(
                                    nT[:, j0:j0 + 4, i * P:(i + 1) * P], tp[:])

                    def emit_proj(g):
                        c0, c1 = g * 512, (g + 1) * 512
                        # kTg for this column group (bf16)
                        for m in range(NJ):
                            pk = apj.tile([P, 512], F32, tag="pj")
                            for j in range(NJ):
                                nc.tensor.matmul(
                                    pk[:], Gw_t[:, j, m * P:(m + 1) * P],
                                    nT[:, j, c0:c1],
                                    start=(j == 0), stop=(j == NJ - 1))
                            nc.vector.tensor_copy(kTg[:, m, c0:c1], pk[:])
                        if g < 2:
                            # gate for the 4 own stripes of this group
                            for si in range(4 * g, 4 * g + 4):
                                pg = apg.tile([P, 1024], F32, tag="pg")
                                for oc in range(2):
                                    for j in range(NJ):
                                        nc.tensor.matmul(
                                            pg[:, oc * 512:(oc + 1) * 512],
                                            nT[:, j, si * P:(si + 1) * P],
                                            gw_t[:, j, oc * 512:(oc + 1) * 512],
                                            start=(j == 0), stop=(j == NJ - 1))
                                glf = aw.tile([P, H], F32, tag="glf", bufs=2)
                                nc.scalar.activation(glf[:], pg[:], AFT.Copy,
                                                     scale=std_all[:, si:si + 1])
                                nc.vector.tensor_add(glf[:], glf[:], b_bc_t[:])
                                nc.scalar.activation(alpha[:, si, :], glf[:],
                                                     AFT.Sigmoid)
                    emit_stripes(0)
                    for g in range(1, 4):
                        emit_stripes(g)
                        emit_proj(g - 1)
                    emit_proj(3)
                    # memory kTg columns
                    for m in range(NJ):
                        pkm = apj.tile([P, T_MEM], F32, tag="pj")
                        for j in range(NJ):
                            nc.tensor.matmul(
                                pkm[:], Gw_t[:, j, m * P:(m + 1) * P],
                                nT[:, j, 2048:TKV],
                                start=(j == 0), stop=(j == NJ - 1))
                        nc.vector.tensor_copy(kTg[:, m, 2048:TKV], pkm[:])

            # ============ phase B: attention
            with tc.tile_pool(name="bw", bufs=1) as bw, \
                 tc.tile_pool(name="bexp", bufs=3) as bexp, \
                 tc.tile_pool(name="bps", bufs=2, space="PSUM") as bps, \
                 tc.tile_pool(name="bph", bufs=1, space="PSUM") as bph:
                for k in range(4):
                    q0 = k * 256
                    ph = [bph.tile([P, 1024], F32, tag=f"ph{sl}",
                                   name=f"ph{sl}") for sl in range(2)]
                    pd = [bph.tile([P, 1], F32, tag=f"pd{sl}", name=f"pd{sl}")
                          for sl in range(2)]
                    # jobs: own pairs, other pairs, then memory
                    jobs = ([("own", 2 * pi) for pi in range(k + 1)]
                            + [("oth", 8 + 2 * pi) for pi in range(k + 1)]
                            + [("mem", 16)])
                    ets = []

                    def emit_scores(ji):
                        kind, pos = jobs[ji]
                        if kind == "mem":
                            et = bexp.tile([T_MEM, 256], BF16, tag="etm")
                            ps = bps.tile([P, 512], F32, tag="ps")
                            for j in range(NJ):
                                nc.tensor.matmul(
                                    ps[:T_MEM, 0:256],
                                    kTg[:, j, 2048:TKV],
                                    nT[:, j, q0:q0 + 256],
                                    start=(j == 0), stop=(j == NJ - 1))
                            nc.scalar.activation(et[:], ps[:T_MEM, 0:256],
                                                 AFT.Exp, scale=SCALE,
                                                 bias=shift_t[:T_MEM, :])
                            ets.append(et)
                            return
                        et = bexp.tile([P, 2, 256], FP8, tag="et")
                        ps = bps.tile([P, 512], F32, tag="ps")
                        for ko in range(2):
                            t = pos + ko
                            for j in range(NJ):
                                nc.tensor.matmul(
                                    ps[:, ko * 256:(ko + 1) * 256],
                                    kTg[:, j, t * P:(t + 1) * P],
                                    nT[:, j, q0:q0 + 256],
                                    start=(j == 0), stop=(j == NJ - 1))
                        if kind == "own" and pos == 2 * k:
                            nc.vector.tensor_add(ps[:], ps[:],
                                                 mask_t[:, 0:512])
                        elif kind == "oth" and pos == 8 + 2 * k:
                            nc.vector.tensor_add(ps[:], ps[:],
                                                 mask_t[:, 512:1024])
                        nc.scalar.activation(et[:, 0:2, :], ps[:], AFT.Exp,
                                             scale=SCALE, bias=shift_t[:])
                        ets.append(et)

                    def emit_hacc(ji):
                        kind, pos = jobs[ji]
                        first = ji == 0
                        last = ji == len(jobs) - 1
                        et = ets[ji]
                        for sl in range(2):
                            if kind == "mem":
                                lt = et[:, sl * P:(sl + 1) * P]
                                for hc in range(2):
                                    nc.tensor.matmul(
                                        ph[sl][:, hc * 512:(hc + 1) * 512], lt,
                                        vmem_t[:, hc * 512:(hc + 1) * 512],
                                        start=first, stop=last,
                                        skip_group_check=True)
                                nc.tensor.matmul(
                                    pd[sl][:], lt, ones16_t[:],
                                    start=first, stop=last,
                                    skip_group_check=True)
                            else:
                                lt = et[:, 0:2, sl * P:(sl + 1) * P]
                                for hc in range(2):
                                    nc.tensor.matmul(
                                        ph[sl][:, hc * 512:(hc + 1) * 512], lt,
                                        v8[:, pos:pos + 2,
                                           hc * 512:(hc + 1) * 512],
                                        start=first, stop=False,
                                        perf_mode=DR, skip_group_check=True)
                                    nc.tensor.matmul(
                                        ph[sl][:, hc * 512:(hc + 1) * 512], lt,
                                        vr8[:, pos:pos + 2,
                                            hc * 512:(hc + 1) * 512],
                                        start=False, stop=False,
                                        perf_mode=DR, skip_group_check=True)
                                nc.tensor.matmul(
                                    pd[sl][:], lt, ones_t[:, 0:2, :],
                                    start=first, stop=False,
                                    perf_mode=DR, skip_group_check=True)

                    for ji in range(len(jobs)):
                        emit_scores(ji)
                        if ji >= 1:
                            emit_hacc(ji - 1)
                    emit_hacc(len(jobs) - 1)

                    for sl in range(2):
                        sidx = 2 * k + sl
                        nc.vector.reciprocal(rden[:, sidx:sidx + 1], pd[sl][:])
                        hsb = bw.tile([P, H], F32, tag="hsb", bufs=2)
                        nc.vector.scalar_tensor_tensor(
                            hsb[:], ph[sl][:], rden[:, sidx:sidx + 1],
                            w_bc_t[:], mybir.AluOpType.mult,
                            mybir.AluOpType.mult)
                        xs = bw.tile([P, H], F32, tag="xs", bufs=2)
                        nc.sync.dma_start(
                            out=xs[:], in_=x_full[sidx * P:(sidx + 1) * P, :])
                        a32 = bw.tile([P, H], F32, tag="a32", bufs=2)
                        nc.scalar.activation(a32[:], alpha[:, sidx, :],
                                             AFT.Copy)
                        nc.vector.tensor_sub(hsb[:], hsb[:], xs[:])
                        nc.vector.tensor_mul(hsb[:], hsb[:], a32[:])
                        nc.gpsimd.tensor_add(hsb[:], hsb[:], xs[:])
                        nc.sync.dma_start(out=out[sidx * P:(sidx + 1) * P, :],
                                          in_=hsb[:])

    import os
    if os.environ.get("NO_WAIT_SPLIT") != "1":
        split_multi_waits(nc, limit=1, dma_limit=1)
    return nc


_NC_CACHE = None
_LAST_IN_MAPS = None


def _get_nc():
    global _NC_CACHE
    if _NC_CACHE is None:
        _NC_CACHE = build_nc()
    return _NC_CACHE


def _mk_masks(h):
    tri = np.where(np.arange(P)[None, :] >= np.arange(P)[:, None],
                   np.float32(0.0), np.float32(NEG)).astype(np.float32)
    Z = np.zeros((P, P), np.float32)
    NB = np.full((P, P), NEG, np.float32)
    m0 = np.concatenate([tri, Z], axis=1)
    m1 = np.concatenate([NB, tri], axis=1)
    if h == 0:
        m2 = np.concatenate([NB, Z], axis=1)
        m3 = np.concatenate([NB, NB], axis=1)
    else:
        m2 = np.concatenate([Z, Z], axis=1)
        m3 = np.concatenate([NB, Z], axis=1)
    return np.concatenate([m0, m1, m2, m3], axis=1)


def prepare_in_maps(hidden_states, memory_state, q_w, k_w, norm_w, gate_w,
                    gate_b):
    hidden_states = np.asarray(hidden_states, dtype=np.float32)
    memory_state = np.asarray(memory_state, dtype=np.float32)
    q_w = np.asarray(q_w, dtype=np.float32)
    k_w = np.asarray(k_w, dtype=np.float32)
    norm_w = np.asarray(norm_w, dtype=np.float32)
    gate_w = np.asarray(gate_w, dtype=np.float32)
    gate_b = np.asarray(gate_b, dtype=np.float32)

    def wrearrange(wT):   # [h, o] -> [p, j, o]
        return np.ascontiguousarray(wT.reshape(NJ, P, H).transpose(1, 0, 2))

    G = (q_w * norm_w[None, :]).T @ (k_w * norm_w[None, :])   # [h, h']
    Gw16 = wrearrange(np.ascontiguousarray(G.T)).astype(NP_BF16)
    gw16 = wrearrange(np.ascontiguousarray(gate_w.T)).astype(NP_BF16)
    w_bc32 = np.ascontiguousarray(np.broadcast_to(norm_w, (P, H)))
    b_bc = np.ascontiguousarray(np.broadcast_to(gate_b, (P, H)))
    id16 = np.eye(P, dtype=np.float32).astype(NP_BF16)
    ones8 = np.ones((P, 2, 1), np.float32).astype(NP_FP8)
    safe_nw = np.where(norm_w == 0.0, 1.0, norm_w)

    in_maps = []
    for c in range(N_CORES):
        b, h = divmod(c, 2)
        xs = hidden_states[b].reshape(NS, P, H)
        x_full = np.ascontiguousarray(
            np.concatenate([xs[h::2], xs[1 - h::2]], axis=0).reshape(2048, H))
        memb = memory_state[b]
        memx16 = np.ascontiguousarray(
            (memb / safe_nw[None, :]).T.reshape(NJ, P, T_MEM)
            .transpose(1, 0, 2)).astype(NP_BF16)
        in_maps.append({
            "x_full": x_full,
            "Gw16": Gw16, "gw16": gw16,
            "memx16": memx16,
            "vmem16": np.ascontiguousarray(memb).astype(NP_BF16),
            "masks": _mk_masks(h),
            "w_bc32": w_bc32, "b_bc": b_bc,
            "id16": id16, "ones8": ones8,
        })
    return in_maps


def kernel(**inputs):
    in_maps = prepare_in_maps(**inputs)
    global _LAST_IN_MAPS
    _LAST_IN_MAPS = in_maps
    nc = _get_nc()
    res = run_bass_kernel_spmd(nc, in_maps, list(range(N_CORES)))
    out = np.empty((B_FULL, S_FULL, H), dtype=np.float32)
    for c in range(N_CORES):
        b, h = divmod(c, 2)
        o = res.results[c]["out"].reshape(NJ, P, H)
        for i in range(NJ):
            out[b, (2 * i + h) * P:(2 * i + h + 1) * P] = o[i]
    return out
